# revision 99
# baseline (speedup 1.0000x reference)
"""Transformer block (pre-LN MHA + FFN) Trainium2 Bass kernel.

Data-parallel over 8 cores: core c handles batch b=c//2, sequence half c%2.
Each core computes LN1 + K/V over the batch's FULL 2048 rows (K/V duplicated
across the 2 cores sharing a batch), Q/attention/LN2/FFN over its own 1024
rows.  The core's own half is rolled to columns 0:1024 host-side (attention
is permutation-invariant over the k/v order), so the SPMD program is
identical on all cores.

Device layout: activations transposed [D-partitions, seq-free] throughout.
The attention branch (LN1 stats, Q/K/V projections, scores, probs@V) runs in
fp8e4 with DoubleRow matmuls (2 stacked 128-row contractions per pass);
weights are pre-scaled x32 host-side so fp8 stays in its normal range, and
the softmax is exp(s/8)/4 (the /4 guards fp8 overflow and cancels in the
denominator; the x32 of V cancels the same way via a 32-valued ones column).
Scores contract only 64 (head_dim), so the q operand carries a zeroed
DoubleRow pair slot (the K pair slot reads the next k-tile, which those
zeros cancel).  The FFN runs as a 3-term fp8 expansion (x8@W8 + x8r@W8 +
x8@W8r, where *8r are the fp8 residuals of the fp8 quantization) -- as
accurate as bf16 at 3/4 the DoubleRow PE cost; only LN2 and the g2-diag
identity term stay bf16.  Softmax denominators via the appended
ones-column on V; division deferred past attn@V via reciprocal + E-matrix
PE broadcast.  LN gains/biases are folded into the projection weights
host-side; LN2's beta2 rides b1/b2, its g2 the identity slab w2id.

Emission interleaves a filler-unit queue with the attention windows so the
PE stays busy while the ACT engine works through the softmax exps:
attention(qc=0) hides K/V/Q projection units, attention(qc=1) hides the
full FFN of qc=0.

Self-contained: hardcodes shapes B=4, S=2048, D=1024, H=16, FF=4096.
"""

import numpy as np
import ml_dtypes

import concourse.bass as bass
import concourse.bacc as bacc
import concourse.tile as tile
from concourse import mybir

F32 = mybir.dt.float32
BF16 = mybir.dt.bfloat16
FP8 = mybir.dt.float8e4
AF = mybir.ActivationFunctionType
OP = mybir.AluOpType
DRM = mybir.MatmulPerfMode.DoubleRow
I16 = mybir.dt.int16

B, S, D, H, FF = 4, 2048, 1024, 16, 4096
HD = D // H          # 64
P = 128
DT = D // P          # 8  d-tiles
DP = DT // 2         # 4  d-tile pairs (DoubleRow)
FT = FF // P         # 32 ff-tiles
FTA = FT + DT        # 40 ff-tiles with diag(g2) augmentation
KT = S // P          # 16 k-row tiles
KP = KT // 2         # 8  k-tile pairs
SQ = S // 2          # 1024 own q columns per core
NQ = SQ // 512       # 2 q-chunks of 512
NS = S // 512        # 4 s-chunks of 512
EPS = 1e-5
NCORES = 8
WSC = 32.0                        # fp8 projection-weight pre-scale
EXPS = 0.125 / (WSC * WSC)        # exp scale on raw fp8-scaled scores
EXPB = -float(np.log(4.0))        # exp bias: /4 overflow guard (cancels)
# Schraudolph bit-trick exp for the DVE/Pool offload path:
# bf16bits = trunc(A*(EXPS*s + EXPB) + Bcal); bitcast -> approx exp (~1.5%)
SCHA = 184.6650053 * EXPS
SCHB = 16248.25 + 184.6650053 * EXPB

_CACHE = {}
PHASES = []           # (label, first instruction number) — profiling aid


def _mark(nc, label):
    PHASES.append((label, int(nc.get_next_instruction_name()[2:])))


def _build_nc():
    nc = bacc.Bacc("TRN2", target_bir_lowering=False, debug=False,
                   num_devices=NCORES)

    xbf = nc.dram_tensor("xbf", [P, DT, S], BF16, kind="ExternalInput")
    x8 = nc.dram_tensor("x8", [P, DT, S], FP8, kind="ExternalInput")
    xq8 = nc.dram_tensor("xq8", [P, DT, S], FP8, kind="ExternalInput")
    xh = nc.dram_tensor("xh", [P, DT, SQ], F32, kind="ExternalInput")
    wq = nc.dram_tensor("wq", [P, DT, D], FP8, kind="ExternalInput")
    wk = nc.dram_tensor("wk", [P, DT, D], FP8, kind="ExternalInput")
    wv = nc.dram_tensor("wv", [P, DT, D], FP8, kind="ExternalInput")
    w1 = nc.dram_tensor("w1", [P, DT, 2, FF], FP8, kind="ExternalInput")
    w2a = nc.dram_tensor("w2a", [P, FT, 2, D], FP8, kind="ExternalInput")
    w2id = nc.dram_tensor("w2id", [P, DT, P], BF16, kind="ExternalInput")
    bq = nc.dram_tensor("bq", [P, DT], F32, kind="ExternalInput")
    bk = nc.dram_tensor("bk", [P, DT], F32, kind="ExternalInput")
    bvb = nc.dram_tensor("bvb", [P, D], F32, kind="ExternalInput")
    b1 = nc.dram_tensor("b1", [P, FT], F32, kind="ExternalInput")
    b2 = nc.dram_tensor("b2", [P, DT], F32, kind="ExternalInput")
    emat = nc.dram_tensor("emat", [2, DT, P], BF16, kind="ExternalInput")
    OUT = nc.dram_tensor("OUT", [P, DT, SQ], F32, kind="ExternalOutput")

    import os
    repeat = int(os.environ.get("BASS_KERNEL_REPEAT", "1"))
    with tile.TileContext(nc) as tc:
        for _ in range(repeat):
            _emit(nc, tc, xbf, x8, xq8, xh, wq, wk, wv, w1, w2a, w2id,
                  bq, bk, bvb, b1, b2, emat, OUT)
    nc.compile()
    return nc


def _emit(nc, tc, xbf_d, x8_d, xq8_d, xh_d, wq_d, wk_d, wv_d, w1_d, w2a_d,
          w2id_d, bq_d, bk_d, bvb_d, b1_d, b2_d, emat_d, OUT_d):
    pools = {}
    pool_objs = {}

    def open_pool(name, bufs, space="SBUF"):
        cm = tc.tile_pool(name=name, bufs=bufs, space=space)
        pools[name] = cm
        pool_objs[name] = cm.__enter__()
        return pool_objs[name]

    def close_pool(name):
        pools.pop(name).__exit__(None, None, None)

    # ---- pools (LIFO open/close discipline) ----
    p_const = open_pool("consts", 1)
    p_ps = open_pool("psg", 2, space="PSUM")           # [128,512] general
    p_sc = open_pool("scps", 2, space="PSUM")          # [128,2,512] scores
    p_ap = open_pool("attps", 2, space="PSUM")         # [65,512] attn accum
    p_per = open_pool("persist", 1)                    # qT8/kz/vaug8/rgather
    p_ex = open_pool("exu", 2)                         # exp out, per kt-pair
    p_st = open_pool("stage", 2)                       # attn psum staging
    p_at = open_pool("attn", 1)                        # attn_raw per-qc

    # ---- constants ----
    ones_bf = p_const.tile([P, P], BF16, tag="ones")
    nc.vector.memset(ones_bf[:], 1.0)
    ones8 = p_const.tile([P, 2, P], FP8, tag="ones8")
    nc.vector.memset(ones8[:], 1.0)
    eps_t = p_const.tile([P, 1], F32, tag="eps")
    nc.vector.memset(eps_t[:], EPS)
    expb_t = p_const.tile([P, 1], F32, tag="expb")
    nc.vector.memset(expb_t[:], EXPB)
    sb_bq = p_const.tile([P, DT], F32, tag="bq")
    nc.sync.dma_start(sb_bq[:], bq_d[:, :])
    sb_bk = p_const.tile([P, DT], F32, tag="bk")
    nc.sync.dma_start(sb_bk[:], bk_d[:, :])
    sb_bvb = p_const.tile([P, D], F32, tag="bvb")
    nc.sync.dma_start(sb_bvb[:], bvb_d[:, :])
    sb_b1 = p_const.tile([P, FT], F32, tag="b1")
    nc.sync.dma_start(sb_b1[:], b1_d[:, :])
    sb_b2 = p_const.tile([P, DT], F32, tag="b2")
    nc.sync.dma_start(sb_b2[:], b2_d[:, :])
    sb_emat = p_const.tile([2, DT, P], BF16, tag="emat")
    nc.sync.dma_start(sb_emat[:], emat_d[:, :, :])
    sb_w2id = p_const.tile([P, DT, P], BF16, tag="w2id")
    nc.sync.dma_start(sb_w2id[:], w2id_d[:, :, :])

    # ---- persistent attention tensors ----
    # qT8[p, t, qc, 0:512] = q values, [.., 512:1024] = zeros: the zero half
    # is the DoubleRow moving pair slot, so the K-side pair slot (the next
    # k-tile, or the zeroed KT guard tile) contributes exactly zero.
    qT8 = p_per.tile([P, DT, 2, 1024], FP8, tag="qT8")
    kz = p_per.tile([P, DT, KT + 1, P], FP8, tag="kz")
    vaug8 = p_per.tile([P, KT, H, HD + 1], FP8, tag="vaug8")
    rgather_t = {}

    def rgather_for(qc):
        if qc not in rgather_t:
            rgather_t[qc] = p_per.tile([2, DT, 512], BF16, tag="rgather",
                                       name=f"rgather{qc}")
        return rgather_t[qc]

    def xh_load(qc):
        xh_c = p_per.tile([P, DT, 512], F32, tag="xh", name=f"xh{qc}")
        nc.sync.dma_start(xh_c[:], xh_d[:, :, bass.ts(qc, 512)])
        return xh_c

    nc.gpsimd.memset(qT8[:, :, :, 512:1024], 0.0)
    nc.gpsimd.memset(kz[:, :, KT, :], 0.0)             # finite guard tile
    nc.gpsimd.memset(vaug8[:, :, :, HD:HD + 1], WSC)   # ones column = 32

    # =========================================================
    # LN1 over all S columns -> h8 (fp8, [P, DT, S]); per-512-chunk units
    # pipelined into the A0 attention windows.  Stats come from
    # host-shipped fp8 x / x^2 via DoubleRow ones-matmuls.
    # =========================================================
    p_w = open_pool("qkvw", 1)       # wq/wk/wv slabs (A0 fillers)
    p_h8 = open_pool("h8p", 1)
    h8 = p_h8.tile([P, DT, S], FP8, tag="h8")

    p_lx = open_pool("ln1x", 1)
    p_lt = open_pool("ln1tmp", 1)

    xbf = p_lx.tile([P, DT, S], BF16, tag="xbf")
    x8s = p_lx.tile([P, DT, S], FP8, tag="x8s")
    xq8s = p_lx.tile([P, DT, S], FP8, tag="xq8s")
    for sc in range(NS):
        ssl = bass.ts(sc, 512)
        nc.sync.dma_start(x8s[:, :, ssl], x8_d[:, :, ssl])
        nc.sync.dma_start(xq8s[:, :, ssl], xq8_d[:, :, ssl])
        nc.sync.dma_start(xbf[:, :, ssl], xbf_d[:, :, ssl])

    def ln1unit(sc):
        def go():
            ssl = bass.ts(sc, 512)
            pstat = p_sc.tile([P, 2, 512], F32, tag="scps",
                              name=f"ln1stat{sc}")
            for j in range(DP):
                nc.tensor.matmul(pstat[:, 0, :], ones8[:],
                                 x8s[:, 2 * j:2 * j + 2, ssl],
                                 start=(j == 0), stop=(j == DP - 1),
                                 perf_mode=DRM)
            for j in range(DP):
                nc.tensor.matmul(pstat[:, 1, :], ones8[:],
                                 xq8s[:, 2 * j:2 * j + 2, ssl],
                                 start=(j == 0), stop=(j == DP - 1),
                                 perf_mode=DRM)
            mu = p_lt.tile([P, 512], F32, tag="mu")
            nc.scalar.mul(mu[:], pstat[:, 0, :], 1.0 / D)
            msq = p_lt.tile([P, 512], F32, tag="msq")
            nc.scalar.mul(msq[:], pstat[:, 1, :], 1.0 / D)
            var = p_lt.tile([P, 512], F32, tag="var")
            nc.vector.tensor_mul(var[:], mu[:], mu[:])
            nc.vector.tensor_sub(var[:], msq[:], var[:])
            sdv = p_lt.tile([P, 512], F32, tag="sdv")
            nc.scalar.activation(sdv[:], var[:], AF.Sqrt, bias=eps_t[:],
                                 scale=1.0)
            rstd = p_lt.tile([P, 512], BF16, tag="rstd")
            with nc.allow_low_precision(reason="rstd bf16 matches matmuls"):
                nc.vector.reciprocal(rstd[:], sdv[:])
            negmu = p_lt.tile([P, 512], F32, tag="negmu")
            nc.gpsimd.tensor_scalar(negmu[:], mu[:], -1.0, None, OP.mult)
            nsb = p_lt.tile([P, 512], BF16, tag="nsb")
            nc.vector.tensor_mul(nsb[:], negmu[:], rstd[:])
            tmpb = p_lt.tile([P, DT, 512], BF16, tag="tmpb")
            nc.vector.tensor_tensor(
                tmpb[:], xbf[:, :, ssl],
                rstd[:, None, :].to_broadcast((P, DT, 512)), OP.mult)
            nc.vector.tensor_tensor(
                h8[:, :, ssl], tmpb[:],
                nsb[:, None, :].to_broadcast((P, DT, 512)), OP.add)
        return go

    # =========================================================
    # projection units (emitted directly or via the A0 filler queue)
    # =========================================================
    def kunit(t, sc):
        def go():
            if sc == 0:
                wk_s = p_w.tile([P, DT, P], FP8, tag="wk_s")
                nc.sync.dma_start(wk_s[:], wk_d[:, :, bass.ts(t, P)])
                pool_objs["_wk"] = wk_s
            wk_s = pool_objs["_wk"]
            pk = p_ps.tile([P, 512], F32, tag="psg")
            for j in range(DP):
                nc.tensor.matmul(pk[:], wk_s[:, 2 * j:2 * j + 2, :],
                                 h8[:, 2 * j:2 * j + 2, bass.ts(sc, 512)],
                                 start=(j == 0), stop=(j == DP - 1),
                                 perf_mode=DRM)
            nc.vector.tensor_scalar(
                kz[:, t, 4 * sc:4 * sc + 4, :],
                pk[:].rearrange("p (a b) -> p a b", a=4),
                sb_bk[:, t:t + 1], None, OP.add)
        return go

    def qunit(t, qc):
        def go():
            wq_s = p_w.tile([P, DT, P], FP8, tag="wq_s")
            nc.sync.dma_start(wq_s[:], wq_d[:, :, bass.ts(t, P)])
            pq = p_ps.tile([P, 512], F32, tag="psg")
            for j in range(DP):
                nc.tensor.matmul(pq[:], wq_s[:, 2 * j:2 * j + 2, :],
                                 h8[:, 2 * j:2 * j + 2, bass.ts(qc, 512)],
                                 start=(j == 0), stop=(j == DP - 1),
                                 perf_mode=DRM)
            nc.vector.tensor_scalar(qT8[:, t, qc, 0:512], pq[:],
                                    sb_bq[:, t:t + 1], None, OP.add)
        return go

    def vunit(g, kt):
        def go():
            if kt == 0:
                wv_s = p_w.tile([P, DT, 512], FP8, tag="wv_s")
                nc.sync.dma_start(wv_s[:], wv_d[:, :, bass.ts(g, 512)])
                pool_objs["_wv"] = wv_s
            wv_s = pool_objs["_wv"]
            pv = p_ps.tile([P, 512], F32, tag="psg")
            for j in range(DP):
                nc.tensor.matmul(pv[:], h8[:, 2 * j:2 * j + 2, bass.ts(kt, P)],
                                 wv_s[:, 2 * j:2 * j + 2, :],
                                 start=(j == 0), stop=(j == DP - 1),
                                 perf_mode=DRM)
            nc.vector.tensor_tensor(
                vaug8[:, kt, 8 * g:8 * g + 8, 0:HD],
                pv[:].rearrange("p (h d) -> p h d", d=HD),
                sb_bvb[:, bass.ts(g, 512)].rearrange("p (h d) -> p h d", d=HD),
                OP.add)
        return go

    # =========================================================
    # attention step for one (t, qc): scores (DoubleRow, zero-padded K),
    # exp on ACT (fp8 out), probs@V (DoubleRow over kt pairs)
    # =========================================================
    def attention_step(qc, t, drain, expmap=None):
        qsl = bass.ts(qc, 512)
        drain(t, -1)
        aps = [p_ap.tile([HD + 1, 512], F32, tag="attps",
                         name=f"attps_{qc}_{t}_{i}") for i in range(2)]
        exus = {}

        def pv_pair(u):
            exu_t = exus.pop(u)
            for i in range(2):
                nc.tensor.matmul(aps[i][:],
                                 vaug8[:, 2 * u:2 * u + 2, 2 * t + i, :],
                                 exu_t[:, :, i, :],
                                 start=(u == 0), stop=(u == KP - 1),
                                 perf_mode=DRM)

        for u in range(KP):
            exu_t = p_ex.tile([P, 2, 2, 512], FP8, tag="exu")
            exus[u] = exu_t
            for m in range(2):
                kt = 2 * u + m
                scp = p_sc.tile([P, 2, 512], F32, tag="scps")
                for i in range(2):
                    nc.tensor.matmul(
                        scp[:, i, :],
                        kz[64 * i:64 * i + 64, t, kt:kt + 2, :],
                        qT8[64 * i:64 * i + 64, t, qc, :].rearrange(
                            "p (two f) -> p two f", two=2),
                        start=True, stop=True, perf_mode=DRM)
                eng = (expmap or {}).get(kt)
                if eng is None:
                    nc.scalar.activation(exu_t[:, m, :, :], scp[:], AF.Exp,
                                         bias=expb_t[:], scale=EXPS)
                else:
                    # Schraudolph exp: DVE does the (fast) PSUM read so the
                    # scores ring drains at ACT speed; Pool does the SBUF
                    # bitcast convert.
                    xi = p_per.tile(
                        [P, 2, 512], I16, tag=f"xi{kt % 2}",
                        name=f"xi{qc}_{t}_{kt}")
                    nc.vector.tensor_scalar(xi[:], scp[:], SCHA, SCHB,
                                            OP.mult, OP.add)
                    nc.gpsimd.tensor_copy(exu_t[:, m, :, :],
                                          xi[:].bitcast(BF16))
            if u >= 1:
                pv_pair(u - 1)
            drain(t, u)
        pv_pair(KP - 1)
        for i in range(2):
            h = 2 * t + i
            st = p_st.tile([HD + 1, 512], BF16, tag="stage")
            nc.vector.tensor_copy(st[:], aps[i][:])
            nc.sync.dma_start(attn_raw[64 * i:64 * i + 64, t, :], st[0:HD, :])
            nc.sync.dma_start(rgather_for(qc)[i:i + 1, t, :],
                              st[HD:HD + 1, :])

    attn_raw = p_at.tile([P, DT, 512], BF16, tag="attn_raw")

    # ---- filler queue machinery ----
    def make_drain(queue, cum, startreq):
        state = {"done": 0}

        def drain(t, u):
            if u < 0:
                # step preamble: force everything step t's scores need
                tgt = startreq[t]
            else:
                lo = cum[t - 1] if t > 0 else 0
                tgt = lo + ((cum[t] - lo) * (u + 1) + KP - 1) // KP
            while state["done"] < tgt and queue:
                queue.pop(0)()
                state["done"] += 1
        return drain

    # =========================================================
    # A0: attention(qc=0), hiding the remaining K/V/Q units
    # =========================================================
    q0 = [kunit(0, 1)]                                        # 0
    q0 += [vunit(0, kt) for kt in range(4)]                   # 1..4
    q0 += [kunit(0, 2)]                                       # 5
    q0 += [vunit(0, kt) for kt in range(4, 8)]                # 6..9
    q0 += [kunit(0, 3)]                                       # 10
    q0 += [vunit(0, kt) for kt in range(8, 16)]               # 11..18
    q0 += [kunit(1, sc) for sc in range(NS)] + [qunit(1, 0)]  # 19..23
    q0 += [kunit(2, sc) for sc in range(NS)] + [qunit(2, 0)]  # 24..28
    q0 += [kunit(3, sc) for sc in range(NS)] + [qunit(3, 0)]  # 29..33
    q0 += [vunit(1, kt) for kt in range(6)]                   # 34..39
    q0 += [kunit(4, sc) for sc in range(NS)] + [qunit(4, 0)]  # 40..44
    q0 += [vunit(1, kt) for kt in range(6, 12)]               # 45..50
    q0 += [kunit(5, sc) for sc in range(NS)] + [qunit(5, 0)]  # 51..55
    q0 += [vunit(1, kt) for kt in range(12, 16)]              # 56..59
    q0 += [kunit(6, sc) for sc in range(NS)] + [qunit(6, 0)]  # 60..64
    q0 += [kunit(7, sc) for sc in range(NS)] + [qunit(7, 0)]  # 65..69
    q0 += [qunit(t, 1) for t in range(DT)]                    # 70..77

    _mark(nc, "prologue")
    for sc in range(NS):
        ln1unit(sc)()
    kunit(0, 0)()
    qunit(0, 0)()

    close_pool("ln1tmp")
    close_pool("ln1x")

    EXPMAP0 = {3: "x", 7: "x", 11: "x", 15: "x"}
    # cum0[t-1] must cover every unit step t's scores AND pv-pairs read
    # (kz writes for t, vaug writes for heads 2t..2t+1): emission order IS
    # the dataflow order for the tile framework.
    cum0 = [27, 32, 37, 60, 64, 69, 74, 78]
    sreq0 = [0, 24, 29, 34, 60, 64, 69, 74]
    drain0 = make_drain(q0, cum0, sreq0)
    for t in range(DT):
        _mark(nc, f"A0.t{t}")
        attention_step(0, t, drain0, EXPMAP0 if t >= 1 else None)
    while q0:
        q0.pop(0)()

    close_pool("h8p")
    close_pool("qkvw")

    # =========================================================
    # y / LN2 / FFN plumbing
    # =========================================================
    p_y = open_pool("ybfp", 1)
    p_fT = open_pool("fTp", 1)
    p_l2 = open_pool("ln2tmp", 1)
    p_yt = open_pool("ytmp", 1)
    p_f1w = open_pool("w1slab", 3)
    p_f2w = open_pool("w2slab", 2)
    p_fr = open_pool("relu", 1)
    p_fo = open_pool("fout", 2)

    # LN2 output, split as fp8 value + fp8 residual (3-term fp8 FFN)
    f8_t = {qc: p_fT.tile([P, DT, 512], FP8, tag=f"f8{qc}", name=f"f8{qc}")
            for qc in range(NQ)}
    f8r_t = {qc: p_fT.tile([P, DT, 512], FP8, tag=f"f8r{qc}",
                           name=f"f8r{qc}")
             for qc in range(NQ)}
    ybf = p_y.tile([P, DT, 512], BF16, tag="ybf")
    r8 = p_fr.tile([P, FT, 512], FP8, tag="r8")
    r8r = p_fr.tile([P, FT, 512], FP8, tag="r8r")

    def y_ln2(qc):
        """y = attn/denom + x, then LN2 -> f8/f8r.  Per-dt incremental so
        dt-units unblock as soon as attention step t=dt stages its output;
        the LN2 stats accumulate alongside."""
        qsl = bass.ts(qc, 512)
        xh_c = xh_load(qc)
        pstat = p_sc.tile([P, 2, 512], F32, tag="scps", name=f"ln2stat{qc}")
        for dt in range(DT):
            # per-head-pair reciprocal: y(dt) only waits on stage(t=dt)
            rpad = p_yt.tile([2, 512], BF16, tag="rpad",
                             name=f"rpad{qc}_{dt}")
            with nc.allow_low_precision(reason="softmax denom bf16"):
                nc.vector.reciprocal(rpad[:],
                                     rgather_for(qc)[:, dt, :])
            rb = p_ps.tile([P, 512], F32, tag="psg")
            nc.tensor.matmul(rb[:], sb_emat[:, dt, :], rpad[:],
                             start=True, stop=True)
            t1 = p_yt.tile([P, 512], F32, tag="t1")
            nc.vector.tensor_mul(t1[:], attn_raw[:, dt, :], rb[:])
            nc.vector.tensor_add(ybf[:, dt, :], t1[:], xh_c[:, dt, :])
            xsq = p_l2.tile([P, 512], BF16, tag="xsq",
                            name=f"xsq{qc}_{dt}")
            nc.scalar.activation(xsq[:], ybf[:, dt, :], AF.Square)
            nc.tensor.matmul(pstat[:, 0, :], ones_bf[:], ybf[:, dt, :],
                             start=(dt == 0), stop=(dt == DT - 1))
            nc.tensor.matmul(pstat[:, 1, :], ones_bf[:], xsq[:],
                             start=(dt == 0), stop=(dt == DT - 1))
        mu = p_l2.tile([P, 512], F32, tag="mu")
        nc.scalar.mul(mu[:], pstat[:, 0, :], 1.0 / D)
        msq = p_l2.tile([P, 512], F32, tag="msq")
        nc.scalar.mul(msq[:], pstat[:, 1, :], 1.0 / D)
        var = p_l2.tile([P, 512], F32, tag="var")
        nc.vector.tensor_mul(var[:], mu[:], mu[:])
        nc.vector.tensor_sub(var[:], msq[:], var[:])
        sdv = p_l2.tile([P, 512], F32, tag="sdv")
        nc.scalar.activation(sdv[:], var[:], AF.Sqrt, bias=eps_t[:],
                             scale=1.0)
        rstd = p_l2.tile([P, 512], BF16, tag="rstd")
        with nc.allow_low_precision(reason="rstd bf16 matches matmuls"):
            nc.vector.reciprocal(rstd[:], sdv[:])
        negmu = p_l2.tile([P, 512], F32, tag="negmu")
        nc.gpsimd.tensor_scalar(negmu[:], mu[:], -1.0, None, OP.mult)
        nsb = p_l2.tile([P, 512], BF16, tag="nsb")
        nc.vector.tensor_mul(nsb[:], negmu[:], rstd[:])
        for j in range(DP):
            # per dt-pair so FFN1's first DoubleRow pairs unblock early
            jp = slice(2 * j, 2 * j + 2)
            tmpb = p_l2.tile([P, 2, 512], BF16, tag="tmpb",
                             name=f"tmpb{qc}_{j}")
            nc.vector.tensor_tensor(
                tmpb[:], ybf[:, jp, :],
                rstd[:, None, :].to_broadcast((P, 2, 512)), OP.mult)
            fbf = p_l2.tile([P, 2, 512], BF16, tag="fbf",
                            name=f"fbf{qc}_{j}")
            nc.vector.tensor_tensor(
                fbf[:], tmpb[:],
                nsb[:, None, :].to_broadcast((P, 2, 512)), OP.add)
            nc.vector.tensor_copy(f8_t[qc][:, jp, :], fbf[:])
            nc.vector.tensor_tensor(f8r_t[qc][:, jp, :], fbf[:],
                                    f8_t[qc][:, jp, :], OP.subtract)

    def f1unit(qc, ft):
        # pre-act = 32*(f@W1g2) via 3-term fp8: f8@W8 + f8r@W8 + f8@W8r
        # In the tail (qc=1) the attention PSUM pool is idle: alternate
        # accumulators onto it to deepen the unit pipeline.
        def go():
            w1_s = p_f1w.tile([P, DT, 2, P], FP8, tag="w1s")
            nc.sync.dma_start(w1_s[:], w1_d[:, :, :, bass.ts(ft, P)])
            if qc == 1 and ft % 2 == 1:
                pf2 = p_sc.tile([P, 2, 512], F32, tag="scps",
                                name=f"pf2_{qc}_{ft}")
                pf = pf2[:, 0, :]
            else:
                pft = p_ps.tile([P, 512], F32, tag="psg",
                                name=f"pf_{qc}_{ft}")
                pf = pft[:]
            steps = [(0, f8_t[qc]), (1, f8_t[qc]), (0, f8r_t[qc])]
            for si, (r, src) in enumerate(steps):
                for j in range(DP):
                    nc.tensor.matmul(pf, w1_s[:, 2 * j:2 * j + 2, r, :],
                                     src[:, 2 * j:2 * j + 2, :],
                                     start=(si == 0 and j == 0),
                                     stop=(si == 2 and j == DP - 1),
                                     perf_mode=DRM)
            nc.vector.tensor_scalar(r8[:, ft, :], pf,
                                    sb_b1[:, ft:ft + 1], 0.0,
                                    OP.add, OP.max)
            rbf = p_yt.tile([P, 512], BF16, tag="rbf")
            if qc == 0:
                nc.vector.tensor_scalar(rbf[:], pf,
                                        sb_b1[:, ft:ft + 1], 0.0,
                                        OP.add, OP.max)
            else:
                nc.scalar.activation(rbf[:], pf, AF.Relu,
                                     bias=sb_b1[:, ft:ft + 1], scale=1.0)
            nc.gpsimd.tensor_tensor(r8r[:, ft, :], rbf[:], r8[:, ft, :],
                                    OP.subtract)
        return go

    def f2unit(qc, mt):
        # out*2048 = r8@W2_8 + r8r@W2_8 + r8@W2r_8 + w2id@(f8 + f8r)
        def go():
            slab = p_f2w.tile([P, FT, 2, P], FP8, tag="w2s")
            for dq in range(4):
                nc.sync.dma_start(slab[:, 8 * dq:8 * dq + 8, :, :],
                                  w2a_d[:, 8 * dq:8 * dq + 8, :,
                                        bass.ts(mt, P)])
            if qc == 1 and mt % 2 == 1:
                po2 = p_sc.tile([P, 2, 512], F32, tag="scps",
                                name=f"po2_{qc}_{mt}")
                po = po2[:, 0, :]
            else:
                pot = p_ps.tile([P, 512], F32, tag="psg",
                                name=f"po_{qc}_{mt}")
                po = pot[:]
            steps = [(0, r8), (1, r8), (0, r8r)]
            for si, (r, src) in enumerate(steps):
                for u in range(FT // 2):
                    nc.tensor.matmul(po, slab[:, 2 * u:2 * u + 2, r, :],
                                     src[:, 2 * u:2 * u + 2, :],
                                     start=(si == 0 and u == 0), stop=False,
                                     perf_mode=DRM)
            nc.tensor.matmul(po, sb_w2id[:, mt, :], f8_t[qc][:, mt, :],
                             start=False, stop=False)
            nc.tensor.matmul(po, sb_w2id[:, mt, :], f8r_t[qc][:, mt, :],
                             start=False, stop=True)
            ot = p_fo.tile([P, 512], F32, tag="ot")
            nc.vector.tensor_scalar(ot[:], po, 1.0 / 2048.0,
                                    sb_b2[:, mt:mt + 1], OP.mult, OP.add)
            nc.sync.dma_start(OUT_d[:, mt, bass.ts(qc, 512)], ot[:])
        return go

    _mark(nc, "y0/ln2")
    y_ln2(0)

    # =========================================================
    # A1: attention(qc=1), hiding the full FFN of qc=0
    # =========================================================
    q1 = [f1unit(0, ft) for ft in range(FT)] + [f2unit(0, mt)
                                               for mt in range(DT)]
    cum1 = [6, 12, 18, 24, 30, 34, 37, 40]
    sreq1 = [0] * DT
    drain1 = make_drain(q1, cum1, sreq1)
    for t in range(DT):
        _mark(nc, f"A1.t{t}")
        attention_step(1, t, drain1)
    while q1:
        q1.pop(0)()

    _mark(nc, "y1/ln2")
    y_ln2(1)
    _mark(nc, "ffn1-tail")
    for ft in range(FT):
        f1unit(1, ft)()
    _mark(nc, "ffn2-tail")
    for mt in range(DT):
        f2unit(1, mt)()
    _mark(nc, "end")

    for name in list(pools)[::-1]:
        close_pool(name)


def _prep_shared(inputs):
    """Host-side weight preprocessing (shared across cores)."""
    f32 = np.float32
    g1 = np.asarray(inputs["g1"], f32)
    beta1 = np.asarray(inputs["beta1"], f32)
    g2 = np.asarray(inputs["g2"], f32)
    beta2 = np.asarray(inputs["beta2"], f32)
    Wq = np.asarray(inputs["Wq"], f32)
    Wk = np.asarray(inputs["Wk"], f32)
    Wv = np.asarray(inputs["Wv"], f32)
    W1 = np.asarray(inputs["W1"], f32)
    W2 = np.asarray(inputs["W2"], f32)

    def fold(Wm, bm):
        Wp = Wm * g1[:, None]
        bp = np.asarray(inputs[bm], f32) + beta1 @ Wm
        return Wp, bp

    Wqp, bqp = fold(Wq, "bq")
    Wkp, bkp = fold(Wk, "bk")
    Wvp, bvp = fold(Wv, "bv")
    W1p = W1 * g2[:, None]
    b1p = np.asarray(inputs["b1"], f32) + beta2 @ W1
    b2p = np.asarray(inputs["b2"], f32) + beta2

    bf = ml_dtypes.bfloat16
    f8 = ml_dtypes.float8_e4m3

    def wtile(Wm, ntile, dtype):
        return np.ascontiguousarray(
            Wm.reshape(ntile, P, Wm.shape[1]).transpose(1, 0, 2)).astype(dtype)

    def wtile_split(Wm, ntile, scale):
        """fp8 value + fp8 residual of scale*Wm, packed [P, ntile, 2, cols]."""
        ws = (scale * Wm).astype(f32)
        a = ws.astype(f8)
        r = (ws - a.astype(f32)).astype(f8)
        at = wtile(a.astype(f32), ntile, f8)
        rt = wtile(r.astype(f32), ntile, f8)
        return np.ascontiguousarray(np.stack([at, rt], axis=2))

    def btile(bv_, ntile):
        return np.ascontiguousarray(bv_.reshape(ntile, P).T).astype(f32)

    E = np.zeros((2, DT, P), f32)
    for t in range(DT):
        for m in range(P):
            E[m // HD, t, m] = 1.0
    E = E.astype(bf)

    # identity (g2-diag) FFN2 term at the 32*64 combined scale, bf16
    w2id = np.zeros((P, DT, P), f32)
    for mt in range(DT):
        for p in range(P):
            w2id[p, mt, p] = 2048.0 * g2[mt * P + p]
    w2id = w2id.astype(bf)

    return {
        "wq": wtile(WSC * Wqp, DT, f8), "wk": wtile(WSC * Wkp, DT, f8),
        "wv": wtile(WSC * Wvp, DT, f8),
        "w1": wtile_split(W1p, DT, WSC),
        "w2a": wtile_split(W2, FT, 64.0),
        "w2id": w2id,
        "bq": btile(WSC * bqp, DT), "bk": btile(WSC * bkp, DT),
        "bvb": np.ascontiguousarray(
            np.broadcast_to(WSC * bvp, (P, D))).astype(f32),
        "b1": btile(WSC * b1p, FT), "b2": btile(b2p, DT),
        "emat": E,
    }


def _per_core_inputs(inputs, shared):
    x = np.asarray(inputs["x"], np.float32)
    f8 = ml_dtypes.float8_e4m3
    maps = []
    for c in range(NCORES):
        b, hf = c // 2, c % 2
        xTn = x[b].T.reshape(DT, P, S).transpose(1, 0, 2)
        if hf == 1:
            # roll so this core's own half is always columns 0:SQ
            xTn = np.concatenate([xTn[:, :, SQ:], xTn[:, :, :SQ]], axis=2)
        xTn = np.ascontiguousarray(xTn)
        m = dict(shared)
        m["xbf"] = xTn.astype(ml_dtypes.bfloat16)
        m["x8"] = xTn.astype(f8)
        m["xq8"] = (xTn.astype(np.float64) ** 2).astype(f8)
        m["xh"] = np.ascontiguousarray(xTn[:, :, :SQ])
        maps.append(m)
    return maps


def _get_sharded():
    """Build (once) the nc + jitted shard_map executable."""
    if "sharded" in _CACHE:
        return _CACHE["sharded"]

    import jax
    from jax.sharding import Mesh, PartitionSpec
    from jax.experimental.shard_map import shard_map
    from concourse import bass2jax
    from concourse import mybir as _mybir

    bass2jax.install_neuronx_cc_hook()
    nc = _build_nc()

    partition_name = (nc.partition_id_tensor.name
                      if nc.partition_id_tensor else None)
    in_names, out_names, out_avals, zero_shapes = [], [], [], []
    for alloc in nc.m.functions[0].allocations:
        if not isinstance(alloc, _mybir.MemoryLocationSet):
            continue
        name = alloc.memorylocations[0].name
        if alloc.kind == "ExternalInput":
            if name != partition_name:
                in_names.append(name)
        elif alloc.kind == "ExternalOutput":
            shape = tuple(alloc.tensor_shape)
            dtype = _mybir.dt.np(alloc.dtype)
            out_names.append(name)
            out_avals.append(jax.core.ShapedArray(shape, dtype))
            zero_shapes.append((shape, dtype))
    n_params = len(in_names)
    all_names = in_names + out_names
    if partition_name is not None:
        all_names = all_names + [partition_name]
    donate = tuple(range(n_params, n_params + len(out_names)))

    def _body(*args):
        operands = list(args)
        if partition_name is not None:
            operands.append(bass2jax.partition_id_tensor())
        outs = bass2jax._bass_exec_p.bind(
            *operands,
            out_avals=tuple(out_avals),
            in_names=tuple(all_names),
            out_names=tuple(out_names),
            lowering_input_output_aliases=(),
            sim_require_finite=True,
            sim_require_nnan=True,
            nc=nc,
        )
        return tuple(outs)

    devices = jax.devices()[:NCORES]
    mesh = Mesh(np.asarray(devices), ("core",))
    nin = n_params + len(out_names)
    sharded = jax.jit(
        shard_map(_body, mesh=mesh,
                  in_specs=(PartitionSpec("core"),) * nin,
                  out_specs=(PartitionSpec("core"),) * len(out_names),
                  check_rep=False),
        donate_argnums=donate, keep_unused=True)

    _CACHE["sharded"] = (nc, sharded, in_names, out_names, out_avals,
                         zero_shapes)
    return _CACHE["sharded"]


def _concat_inputs(in_maps):
    _, _, in_names, _, _, zero_shapes = _get_sharded()
    concat_in = [
        np.concatenate([np.asarray(in_maps[c][n]) for c in range(NCORES)],
                       axis=0)
        for n in in_names
    ]
    concat_zeros = [
        np.zeros((NCORES * s[0], *s[1:]), d) for (s, d) in zero_shapes
    ]
    return concat_in, concat_zeros


def _run(in_maps):
    nc, fn, in_names, out_names, out_avals, zero_shapes = _get_sharded()
    concat_in, concat_zeros = _concat_inputs(in_maps)
    outs = fn(*concat_in, *concat_zeros)
    res = []
    for c in range(NCORES):
        res.append({
            name: np.asarray(outs[i]).reshape(NCORES, *out_avals[i].shape)[c]
            for i, name in enumerate(out_names)
        })
    return res


def kernel(**inputs):
    shared = _prep_shared(inputs)
    in_maps = _per_core_inputs(inputs, shared)
    res = _run(in_maps)
    out = np.empty((B, S, D), np.float32)
    for c in range(NCORES):
        b, hf = c // 2, c % 2
        o = res[c]["OUT"]                       # [P, DT, SQ]
        out[b, hf * SQ:(hf + 1) * SQ, :] = o.transpose(2, 1, 0).reshape(SQ, D)
    return out


# revision 102
# speedup vs baseline: 1.0009x; 1.0009x over previous
"""Transformer block (pre-LN MHA + FFN) Trainium2 Bass kernel.

Data-parallel over 8 cores: core c handles batch b=c//2, sequence half c%2.
Each core computes LN1 + K/V over the batch's FULL 2048 rows (K/V duplicated
across the 2 cores sharing a batch), Q/attention/LN2/FFN over its own 1024
rows.  The core's own half is rolled to columns 0:1024 host-side (attention
is permutation-invariant over the k/v order), so the SPMD program is
identical on all cores.

Device layout: activations transposed [D-partitions, seq-free] throughout.
The attention branch (LN1 stats, Q/K/V projections, scores, probs@V) runs in
fp8e4 with DoubleRow matmuls (2 stacked 128-row contractions per pass);
weights are pre-scaled x32 host-side so fp8 stays in its normal range, and
the softmax is exp(s/8)/4 (the /4 guards fp8 overflow and cancels in the
denominator; the x32 of V cancels the same way via a 32-valued ones column).
Scores contract only 64 (head_dim), so the q operand carries a zeroed
DoubleRow pair slot (the K pair slot reads the next k-tile, which those
zeros cancel).  The FFN runs as a 3-term fp8 expansion (x8@W8 + x8r@W8 +
x8@W8r, where *8r are the fp8 residuals of the fp8 quantization) -- as
accurate as bf16 at 3/4 the DoubleRow PE cost; only LN2 and the g2-diag
identity term stay bf16.  Softmax denominators via the appended
ones-column on V; division deferred past attn@V via reciprocal + E-matrix
PE broadcast.  LN gains/biases are folded into the projection weights
host-side; LN2's beta2 rides b1/b2, its g2 the identity slab w2id.

Emission interleaves a filler-unit queue with the attention windows so the
PE stays busy while the ACT engine works through the softmax exps:
attention(qc=0) hides K/V/Q projection units, attention(qc=1) hides the
full FFN of qc=0.

Self-contained: hardcodes shapes B=4, S=2048, D=1024, H=16, FF=4096.
"""

import numpy as np
import ml_dtypes

import concourse.bass as bass
import concourse.bacc as bacc
import concourse.tile as tile
from concourse import mybir

F32 = mybir.dt.float32
BF16 = mybir.dt.bfloat16
FP8 = mybir.dt.float8e4
AF = mybir.ActivationFunctionType
OP = mybir.AluOpType
DRM = mybir.MatmulPerfMode.DoubleRow
I16 = mybir.dt.int16

B, S, D, H, FF = 4, 2048, 1024, 16, 4096
HD = D // H          # 64
P = 128
DT = D // P          # 8  d-tiles
DP = DT // 2         # 4  d-tile pairs (DoubleRow)
FT = FF // P         # 32 ff-tiles
FTA = FT + DT        # 40 ff-tiles with diag(g2) augmentation
KT = S // P          # 16 k-row tiles
KP = KT // 2         # 8  k-tile pairs
SQ = S // 2          # 1024 own q columns per core
NQ = SQ // 512       # 2 q-chunks of 512
NS = S // 512        # 4 s-chunks of 512
EPS = 1e-5
NCORES = 8
WSC = 32.0                        # fp8 projection-weight pre-scale
EXPS = 0.125 / (WSC * WSC)        # exp scale on raw fp8-scaled scores
EXPB = -float(np.log(4.0))        # exp bias: /4 overflow guard (cancels)
# Schraudolph bit-trick exp for the DVE/Pool offload path:
# bf16bits = trunc(A*(EXPS*s + EXPB) + Bcal); bitcast -> approx exp (~1.5%)
SCHA = 184.6650053 * EXPS
SCHB = 16248.25 + 184.6650053 * EXPB

_CACHE = {}
PHASES = []           # (label, first instruction number) — profiling aid


def _mark(nc, label):
    PHASES.append((label, int(nc.get_next_instruction_name()[2:])))


def _build_nc():
    nc = bacc.Bacc("TRN2", target_bir_lowering=False, debug=False,
                   num_devices=NCORES)

    xbf = nc.dram_tensor("xbf", [P, DT, S], BF16, kind="ExternalInput")
    x8 = nc.dram_tensor("x8", [P, DT, S], FP8, kind="ExternalInput")
    xq8 = nc.dram_tensor("xq8", [P, DT, S], FP8, kind="ExternalInput")
    xh = nc.dram_tensor("xh", [P, DT, SQ], F32, kind="ExternalInput")
    wq = nc.dram_tensor("wq", [P, DT, D], FP8, kind="ExternalInput")
    wk = nc.dram_tensor("wk", [P, DT, D], FP8, kind="ExternalInput")
    wv = nc.dram_tensor("wv", [P, DT, D], FP8, kind="ExternalInput")
    w1 = nc.dram_tensor("w1", [P, DT, 2, FF], FP8, kind="ExternalInput")
    w2a = nc.dram_tensor("w2a", [P, FT, 2, D], FP8, kind="ExternalInput")
    w2id = nc.dram_tensor("w2id", [P, DT, P], BF16, kind="ExternalInput")
    bq = nc.dram_tensor("bq", [P, DT], F32, kind="ExternalInput")
    bk = nc.dram_tensor("bk", [P, DT], F32, kind="ExternalInput")
    bvb = nc.dram_tensor("bvb", [P, D], F32, kind="ExternalInput")
    b1 = nc.dram_tensor("b1", [P, FT], F32, kind="ExternalInput")
    b2 = nc.dram_tensor("b2", [P, DT], F32, kind="ExternalInput")
    emat = nc.dram_tensor("emat", [2, DT, P], BF16, kind="ExternalInput")
    OUT = nc.dram_tensor("OUT", [P, DT, SQ], F32, kind="ExternalOutput")

    import os
    repeat = int(os.environ.get("BASS_KERNEL_REPEAT", "1"))
    with tile.TileContext(nc) as tc:
        for _ in range(repeat):
            _emit(nc, tc, xbf, x8, xq8, xh, wq, wk, wv, w1, w2a, w2id,
                  bq, bk, bvb, b1, b2, emat, OUT)
    nc.compile()
    return nc


def _emit(nc, tc, xbf_d, x8_d, xq8_d, xh_d, wq_d, wk_d, wv_d, w1_d, w2a_d,
          w2id_d, bq_d, bk_d, bvb_d, b1_d, b2_d, emat_d, OUT_d):
    pools = {}
    pool_objs = {}

    def open_pool(name, bufs, space="SBUF"):
        cm = tc.tile_pool(name=name, bufs=bufs, space=space)
        pools[name] = cm
        pool_objs[name] = cm.__enter__()
        return pool_objs[name]

    def close_pool(name):
        pools.pop(name).__exit__(None, None, None)

    # ---- pools (LIFO open/close discipline) ----
    p_const = open_pool("consts", 1)
    p_ps = open_pool("psg", 2, space="PSUM")           # [128,512] general
    p_sc = open_pool("scps", 2, space="PSUM")          # [128,2,512] scores
    p_ap = open_pool("attps", 2, space="PSUM")         # [65,512] attn accum
    p_per = open_pool("persist", 1)                    # qT8/kz/vaug8/rgather
    p_ex = open_pool("exu", 2)                         # exp out, per kt-pair
    p_st = open_pool("stage", 2)                       # attn psum staging
    p_at = open_pool("attn", 1)                        # attn_raw per-qc

    # ---- constants ----
    ones_bf = p_const.tile([P, P], BF16, tag="ones")
    nc.vector.memset(ones_bf[:], 1.0)
    ones8 = p_const.tile([P, 2, P], FP8, tag="ones8")
    nc.vector.memset(ones8[:], 1.0)
    eps_t = p_const.tile([P, 1], F32, tag="eps")
    nc.vector.memset(eps_t[:], EPS)
    expb_t = p_const.tile([P, 1], F32, tag="expb")
    nc.vector.memset(expb_t[:], EXPB)
    sb_bq = p_const.tile([P, DT], F32, tag="bq")
    nc.sync.dma_start(sb_bq[:], bq_d[:, :])
    sb_bk = p_const.tile([P, DT], F32, tag="bk")
    nc.sync.dma_start(sb_bk[:], bk_d[:, :])
    sb_bvb = p_const.tile([P, D], F32, tag="bvb")
    nc.sync.dma_start(sb_bvb[:], bvb_d[:, :])
    sb_b1 = p_const.tile([P, FT], F32, tag="b1")
    nc.sync.dma_start(sb_b1[:], b1_d[:, :])
    sb_b2 = p_const.tile([P, DT], F32, tag="b2")
    nc.sync.dma_start(sb_b2[:], b2_d[:, :])
    sb_emat = p_const.tile([2, DT, P], BF16, tag="emat")
    nc.sync.dma_start(sb_emat[:], emat_d[:, :, :])
    sb_w2id = p_const.tile([P, DT, P], BF16, tag="w2id")
    nc.sync.dma_start(sb_w2id[:], w2id_d[:, :, :])

    # ---- persistent attention tensors ----
    # qT8[p, t, qc, 0:512] = q values, [.., 512:1024] = zeros: the zero half
    # is the DoubleRow moving pair slot, so the K-side pair slot (the next
    # k-tile, or the zeroed KT guard tile) contributes exactly zero.
    qT8 = p_per.tile([P, DT, 2, 1024], FP8, tag="qT8")
    kz = p_per.tile([P, DT, KT + 1, P], FP8, tag="kz")
    vaug8 = p_per.tile([P, KT, H, HD + 1], FP8, tag="vaug8")
    rgather_t = {}

    def rgather_for(qc):
        if qc not in rgather_t:
            rgather_t[qc] = p_per.tile([2, DT, 512], BF16, tag="rgather",
                                       name=f"rgather{qc}")
        return rgather_t[qc]

    def xh_load(qc):
        xh_c = p_per.tile([P, DT, 512], F32, tag="xh", name=f"xh{qc}")
        nc.sync.dma_start(xh_c[:], xh_d[:, :, bass.ts(qc, 512)])
        return xh_c

    nc.gpsimd.memset(qT8[:, :, :, 512:1024], 0.0)
    nc.gpsimd.memset(kz[:, :, KT, :], 0.0)             # finite guard tile
    nc.gpsimd.memset(vaug8[:, :, :, HD:HD + 1], WSC)   # ones column = 32

    # =========================================================
    # LN1 over all S columns -> h8 (fp8, [P, DT, S]); per-512-chunk units
    # pipelined into the A0 attention windows.  Stats come from
    # host-shipped fp8 x / x^2 via DoubleRow ones-matmuls.
    # =========================================================
    p_w = open_pool("qkvw", 1)       # wq/wk/wv slabs (A0 fillers)
    p_h8 = open_pool("h8p", 1)
    h8 = p_h8.tile([P, DT, S], FP8, tag="h8")

    p_lx = open_pool("ln1x", 1)
    p_lt = open_pool("ln1tmp", 1)

    xbf = p_lx.tile([P, DT, S], BF16, tag="xbf")
    x8s = p_lx.tile([P, DT, S], FP8, tag="x8s")
    xq8s = p_lx.tile([P, DT, S], FP8, tag="xq8s")
    for sc in range(NS):
        ssl = bass.ts(sc, 512)
        nc.sync.dma_start(x8s[:, :, ssl], x8_d[:, :, ssl])
        nc.sync.dma_start(xq8s[:, :, ssl], xq8_d[:, :, ssl])
        nc.sync.dma_start(xbf[:, :, ssl], xbf_d[:, :, ssl])

    def ln1unit(sc):
        def go():
            ssl = bass.ts(sc, 512)
            pstat = p_sc.tile([P, 2, 512], F32, tag="scps",
                              name=f"ln1stat{sc}")
            for j in range(DP):
                nc.tensor.matmul(pstat[:, 0, :], ones8[:],
                                 x8s[:, 2 * j:2 * j + 2, ssl],
                                 start=(j == 0), stop=(j == DP - 1),
                                 perf_mode=DRM)
            for j in range(DP):
                nc.tensor.matmul(pstat[:, 1, :], ones8[:],
                                 xq8s[:, 2 * j:2 * j + 2, ssl],
                                 start=(j == 0), stop=(j == DP - 1),
                                 perf_mode=DRM)
            mu = p_lt.tile([P, 512], F32, tag="mu")
            nc.scalar.mul(mu[:], pstat[:, 0, :], 1.0 / D)
            msq = p_lt.tile([P, 512], F32, tag="msq")
            nc.scalar.mul(msq[:], pstat[:, 1, :], 1.0 / D)
            var = p_lt.tile([P, 512], F32, tag="var")
            nc.vector.tensor_mul(var[:], mu[:], mu[:])
            nc.vector.tensor_sub(var[:], msq[:], var[:])
            sdv = p_lt.tile([P, 512], F32, tag="sdv")
            nc.scalar.activation(sdv[:], var[:], AF.Sqrt, bias=eps_t[:],
                                 scale=1.0)
            rstd = p_lt.tile([P, 512], BF16, tag="rstd")
            with nc.allow_low_precision(reason="rstd bf16 matches matmuls"):
                nc.vector.reciprocal(rstd[:], sdv[:])
            negmu = p_lt.tile([P, 512], F32, tag="negmu")
            nc.gpsimd.tensor_scalar(negmu[:], mu[:], -1.0, None, OP.mult)
            nsb = p_lt.tile([P, 512], BF16, tag="nsb")
            nc.vector.tensor_mul(nsb[:], negmu[:], rstd[:])
            tmpb = p_lt.tile([P, DT, 512], BF16, tag="tmpb")
            nc.vector.tensor_tensor(
                tmpb[:], xbf[:, :, ssl],
                rstd[:, None, :].to_broadcast((P, DT, 512)), OP.mult)
            nc.vector.tensor_tensor(
                h8[:, :, ssl], tmpb[:],
                nsb[:, None, :].to_broadcast((P, DT, 512)), OP.add)
        return go

    # =========================================================
    # projection units (emitted directly or via the A0 filler queue)
    # =========================================================
    def kunit(t, sc):
        def go():
            if sc == 0:
                wk_s = p_w.tile([P, DT, P], FP8, tag="wk_s")
                nc.sync.dma_start(wk_s[:], wk_d[:, :, bass.ts(t, P)])
                pool_objs["_wk"] = wk_s
            wk_s = pool_objs["_wk"]
            pk = p_ps.tile([P, 512], F32, tag="psg")
            for j in range(DP):
                nc.tensor.matmul(pk[:], wk_s[:, 2 * j:2 * j + 2, :],
                                 h8[:, 2 * j:2 * j + 2, bass.ts(sc, 512)],
                                 start=(j == 0), stop=(j == DP - 1),
                                 perf_mode=DRM)
            nc.vector.tensor_scalar(
                kz[:, t, 4 * sc:4 * sc + 4, :],
                pk[:].rearrange("p (a b) -> p a b", a=4),
                sb_bk[:, t:t + 1], None, OP.add)
        return go

    def qunit(t, qc):
        def go():
            wq_s = p_w.tile([P, DT, P], FP8, tag="wq_s")
            nc.sync.dma_start(wq_s[:], wq_d[:, :, bass.ts(t, P)])
            pq = p_ps.tile([P, 512], F32, tag="psg")
            for j in range(DP):
                nc.tensor.matmul(pq[:], wq_s[:, 2 * j:2 * j + 2, :],
                                 h8[:, 2 * j:2 * j + 2, bass.ts(qc, 512)],
                                 start=(j == 0), stop=(j == DP - 1),
                                 perf_mode=DRM)
            nc.vector.tensor_scalar(qT8[:, t, qc, 0:512], pq[:],
                                    sb_bq[:, t:t + 1], None, OP.add)
        return go

    def vunit(g, kt):
        def go():
            if kt == 0:
                wv_s = p_w.tile([P, DT, 512], FP8, tag="wv_s")
                nc.sync.dma_start(wv_s[:], wv_d[:, :, bass.ts(g, 512)])
                pool_objs["_wv"] = wv_s
            wv_s = pool_objs["_wv"]
            pv = p_ps.tile([P, 512], F32, tag="psg")
            for j in range(DP):
                nc.tensor.matmul(pv[:], h8[:, 2 * j:2 * j + 2, bass.ts(kt, P)],
                                 wv_s[:, 2 * j:2 * j + 2, :],
                                 start=(j == 0), stop=(j == DP - 1),
                                 perf_mode=DRM)
            nc.vector.tensor_tensor(
                vaug8[:, kt, 8 * g:8 * g + 8, 0:HD],
                pv[:].rearrange("p (h d) -> p h d", d=HD),
                sb_bvb[:, bass.ts(g, 512)].rearrange("p (h d) -> p h d", d=HD),
                OP.add)
        return go

    # =========================================================
    # attention step for one (t, qc): scores (DoubleRow, zero-padded K),
    # exp on ACT (fp8 out), probs@V (DoubleRow over kt pairs)
    # =========================================================
    def attention_step(qc, t, drain, expmap=None):
        qsl = bass.ts(qc, 512)
        drain(t, -1)
        aps = [p_ap.tile([HD + 1, 512], F32, tag="attps",
                         name=f"attps_{qc}_{t}_{i}") for i in range(2)]
        exus = {}

        def pv_pair(u):
            exu_t = exus.pop(u)
            for i in range(2):
                nc.tensor.matmul(aps[i][:],
                                 vaug8[:, 2 * u:2 * u + 2, 2 * t + i, :],
                                 exu_t[:, :, i, :],
                                 start=(u == 0), stop=(u == KP - 1),
                                 perf_mode=DRM)

        for u in range(KP):
            exu_t = p_ex.tile([P, 2, 2, 512], FP8, tag="exu")
            exus[u] = exu_t
            for m in range(2):
                kt = 2 * u + m
                scp = p_sc.tile([P, 2, 512], F32, tag="scps")
                for i in range(2):
                    nc.tensor.matmul(
                        scp[:, i, :],
                        kz[64 * i:64 * i + 64, t, kt:kt + 2, :],
                        qT8[64 * i:64 * i + 64, t, qc, :].rearrange(
                            "p (two f) -> p two f", two=2),
                        start=True, stop=True, perf_mode=DRM)
                eng = (expmap or {}).get(kt)
                if eng is None:
                    nc.scalar.activation(exu_t[:, m, :, :], scp[:], AF.Exp,
                                         bias=expb_t[:], scale=EXPS)
                else:
                    # Schraudolph exp: DVE does the (fast) PSUM read so the
                    # scores ring drains at ACT speed; Pool does the SBUF
                    # bitcast convert.
                    xi = p_per.tile(
                        [P, 2, 512], I16, tag=f"xi{kt % 2}",
                        name=f"xi{qc}_{t}_{kt}")
                    nc.vector.tensor_scalar(xi[:], scp[:], SCHA, SCHB,
                                            OP.mult, OP.add)
                    nc.gpsimd.tensor_copy(exu_t[:, m, :, :],
                                          xi[:].bitcast(BF16))
            if u >= 1:
                pv_pair(u - 1)
            drain(t, u)
        pv_pair(KP - 1)
        for i in range(2):
            h = 2 * t + i
            st = p_st.tile([HD + 1, 512], BF16, tag="stage")
            nc.vector.tensor_copy(st[:], aps[i][:])
            nc.sync.dma_start(attn_raw[64 * i:64 * i + 64, t, :], st[0:HD, :])
            nc.sync.dma_start(rgather_for(qc)[i:i + 1, t, :],
                              st[HD:HD + 1, :])

    attn_raw = p_at.tile([P, DT, 512], BF16, tag="attn_raw")

    # ---- filler queue machinery ----
    def make_drain(queue, cum, startreq):
        state = {"done": 0}

        def drain(t, u):
            if u < 0:
                # step preamble: force everything step t's scores need
                tgt = startreq[t]
            else:
                lo = cum[t - 1] if t > 0 else 0
                tgt = lo + ((cum[t] - lo) * (u + 1) + KP - 1) // KP
            while state["done"] < tgt and queue:
                queue.pop(0)()
                state["done"] += 1
        return drain

    # =========================================================
    # A0: attention(qc=0), hiding the remaining K/V/Q units
    # =========================================================
    q0 = [kunit(0, 1)]                                        # 0
    q0 += [vunit(0, kt) for kt in range(4)]                   # 1..4
    q0 += [kunit(0, 2)]                                       # 5
    q0 += [vunit(0, kt) for kt in range(4, 8)]                # 6..9
    q0 += [kunit(0, 3)]                                       # 10
    q0 += [vunit(0, kt) for kt in range(8, 16)]               # 11..18
    q0 += [kunit(1, sc) for sc in range(NS)] + [qunit(1, 0)]  # 19..23
    q0 += [kunit(2, sc) for sc in range(NS)] + [qunit(2, 0)]  # 24..28
    q0 += [kunit(3, sc) for sc in range(NS)] + [qunit(3, 0)]  # 29..33
    q0 += [vunit(1, kt) for kt in range(6)]                   # 34..39
    q0 += [kunit(4, sc) for sc in range(NS)] + [qunit(4, 0)]  # 40..44
    q0 += [vunit(1, kt) for kt in range(6, 12)]               # 45..50
    q0 += [kunit(5, sc) for sc in range(NS)] + [qunit(5, 0)]  # 51..55
    q0 += [vunit(1, kt) for kt in range(12, 16)]              # 56..59
    q0 += [kunit(6, sc) for sc in range(NS)] + [qunit(6, 0)]  # 60..64
    q0 += [kunit(7, sc) for sc in range(NS)] + [qunit(7, 0)]  # 65..69
    q0 += [qunit(t, 1) for t in range(DT)]                    # 70..77

    _mark(nc, "prologue")
    for sc in range(NS):
        ln1unit(sc)()
    kunit(0, 0)()
    qunit(0, 0)()

    close_pool("ln1tmp")
    close_pool("ln1x")

    EXPMAP0 = {3: "x", 7: "x", 11: "x", 15: "x"}
    # cum0[t-1] must cover every unit step t's scores AND pv-pairs read
    # (kz writes for t, vaug writes for heads 2t..2t+1): emission order IS
    # the dataflow order for the tile framework.
    cum0 = [27, 32, 37, 45, 62, 65, 70, 78]
    sreq0 = [0, 24, 29, 34, 60, 64, 69, 74]
    drain0 = make_drain(q0, cum0, sreq0)
    for t in range(DT):
        _mark(nc, f"A0.t{t}")
        attention_step(0, t, drain0, EXPMAP0 if t >= 1 else None)
    while q0:
        q0.pop(0)()

    close_pool("h8p")
    close_pool("qkvw")

    # =========================================================
    # y / LN2 / FFN plumbing
    # =========================================================
    p_y = open_pool("ybfp", 1)
    p_fT = open_pool("fTp", 1)
    p_l2 = open_pool("ln2tmp", 1)
    p_yt = open_pool("ytmp", 1)
    p_f1w = open_pool("w1slab", 3)
    p_f2w = open_pool("w2slab", 2)
    p_fr = open_pool("relu", 1)
    p_fo = open_pool("fout", 2)

    # LN2 output, split as fp8 value + fp8 residual (3-term fp8 FFN)
    f8_t = {qc: p_fT.tile([P, DT, 512], FP8, tag=f"f8{qc}", name=f"f8{qc}")
            for qc in range(NQ)}
    f8r_t = {qc: p_fT.tile([P, DT, 512], FP8, tag=f"f8r{qc}",
                           name=f"f8r{qc}")
             for qc in range(NQ)}
    ybf = p_y.tile([P, DT, 512], BF16, tag="ybf")
    r8 = p_fr.tile([P, FT, 512], FP8, tag="r8")
    r8r = p_fr.tile([P, FT, 512], FP8, tag="r8r")

    def y_ln2(qc):
        """y = attn/denom + x, then LN2 -> f8/f8r.  Per-dt incremental so
        dt-units unblock as soon as attention step t=dt stages its output;
        the LN2 stats accumulate alongside."""
        qsl = bass.ts(qc, 512)
        xh_c = xh_load(qc)
        pstat = p_sc.tile([P, 2, 512], F32, tag="scps", name=f"ln2stat{qc}")
        for dt in range(DT):
            # per-head-pair reciprocal: y(dt) only waits on stage(t=dt)
            rpad = p_yt.tile([2, 512], BF16, tag="rpad",
                             name=f"rpad{qc}_{dt}")
            with nc.allow_low_precision(reason="softmax denom bf16"):
                nc.vector.reciprocal(rpad[:],
                                     rgather_for(qc)[:, dt, :])
            rb = p_ps.tile([P, 512], F32, tag="psg")
            nc.tensor.matmul(rb[:], sb_emat[:, dt, :], rpad[:],
                             start=True, stop=True)
            t1 = p_yt.tile([P, 512], F32, tag="t1")
            nc.vector.tensor_mul(t1[:], attn_raw[:, dt, :], rb[:])
            nc.vector.tensor_add(ybf[:, dt, :], t1[:], xh_c[:, dt, :])
            xsq = p_l2.tile([P, 512], BF16, tag="xsq",
                            name=f"xsq{qc}_{dt}")
            nc.scalar.activation(xsq[:], ybf[:, dt, :], AF.Square)
            nc.tensor.matmul(pstat[:, 0, :], ones_bf[:], ybf[:, dt, :],
                             start=(dt == 0), stop=(dt == DT - 1))
            nc.tensor.matmul(pstat[:, 1, :], ones_bf[:], xsq[:],
                             start=(dt == 0), stop=(dt == DT - 1))
        mu = p_l2.tile([P, 512], F32, tag="mu")
        nc.scalar.mul(mu[:], pstat[:, 0, :], 1.0 / D)
        msq = p_l2.tile([P, 512], F32, tag="msq")
        nc.scalar.mul(msq[:], pstat[:, 1, :], 1.0 / D)
        var = p_l2.tile([P, 512], F32, tag="var")
        nc.vector.tensor_mul(var[:], mu[:], mu[:])
        nc.vector.tensor_sub(var[:], msq[:], var[:])
        sdv = p_l2.tile([P, 512], F32, tag="sdv")
        nc.scalar.activation(sdv[:], var[:], AF.Sqrt, bias=eps_t[:],
                             scale=1.0)
        rstd = p_l2.tile([P, 512], BF16, tag="rstd")
        with nc.allow_low_precision(reason="rstd bf16 matches matmuls"):
            nc.vector.reciprocal(rstd[:], sdv[:])
        negmu = p_l2.tile([P, 512], F32, tag="negmu")
        nc.gpsimd.tensor_scalar(negmu[:], mu[:], -1.0, None, OP.mult)
        nsb = p_l2.tile([P, 512], BF16, tag="nsb")
        nc.vector.tensor_mul(nsb[:], negmu[:], rstd[:])
        for j in range(DP):
            # per dt-pair so FFN1's first DoubleRow pairs unblock early
            jp = slice(2 * j, 2 * j + 2)
            tmpb = p_l2.tile([P, 2, 512], BF16, tag="tmpb",
                             name=f"tmpb{qc}_{j}")
            nc.vector.tensor_tensor(
                tmpb[:], ybf[:, jp, :],
                rstd[:, None, :].to_broadcast((P, 2, 512)), OP.mult)
            fbf = p_l2.tile([P, 2, 512], BF16, tag="fbf",
                            name=f"fbf{qc}_{j}")
            nc.vector.tensor_tensor(
                fbf[:], tmpb[:],
                nsb[:, None, :].to_broadcast((P, 2, 512)), OP.add)
            nc.vector.tensor_copy(f8_t[qc][:, jp, :], fbf[:])
            nc.vector.tensor_tensor(f8r_t[qc][:, jp, :], fbf[:],
                                    f8_t[qc][:, jp, :], OP.subtract)

    def f1unit(qc, ft):
        # pre-act = 32*(f@W1g2) via 3-term fp8: f8@W8 + f8r@W8 + f8@W8r
        # In the tail (qc=1) the attention PSUM pool is idle: alternate
        # accumulators onto it to deepen the unit pipeline.
        def go():
            w1_s = p_f1w.tile([P, DT, 2, P], FP8, tag="w1s")
            nc.sync.dma_start(w1_s[:], w1_d[:, :, :, bass.ts(ft, P)])
            if qc == 1 and ft % 2 == 1:
                pf2 = p_sc.tile([P, 2, 512], F32, tag="scps",
                                name=f"pf2_{qc}_{ft}")
                pf = pf2[:, 0, :]
            else:
                pft = p_ps.tile([P, 512], F32, tag="psg",
                                name=f"pf_{qc}_{ft}")
                pf = pft[:]
            steps = [(0, f8_t[qc]), (1, f8_t[qc]), (0, f8r_t[qc])]
            for si, (r, src) in enumerate(steps):
                for j in range(DP):
                    nc.tensor.matmul(pf, w1_s[:, 2 * j:2 * j + 2, r, :],
                                     src[:, 2 * j:2 * j + 2, :],
                                     start=(si == 0 and j == 0),
                                     stop=(si == 2 and j == DP - 1),
                                     perf_mode=DRM)
            nc.vector.tensor_scalar(r8[:, ft, :], pf,
                                    sb_b1[:, ft:ft + 1], 0.0,
                                    OP.add, OP.max)
            rbf = p_yt.tile([P, 512], BF16, tag="rbf")
            if qc == 0:
                nc.vector.tensor_scalar(rbf[:], pf,
                                        sb_b1[:, ft:ft + 1], 0.0,
                                        OP.add, OP.max)
            else:
                nc.scalar.activation(rbf[:], pf, AF.Relu,
                                     bias=sb_b1[:, ft:ft + 1], scale=1.0)
            nc.gpsimd.tensor_tensor(r8r[:, ft, :], rbf[:], r8[:, ft, :],
                                    OP.subtract)
        return go

    def f2unit(qc, mt):
        # out*2048 = r8@W2_8 + r8r@W2_8 + r8@W2r_8 + w2id@(f8 + f8r)
        def go():
            slab = p_f2w.tile([P, FT, 2, P], FP8, tag="w2s")
            for dq in range(4):
                nc.sync.dma_start(slab[:, 8 * dq:8 * dq + 8, :, :],
                                  w2a_d[:, 8 * dq:8 * dq + 8, :,
                                        bass.ts(mt, P)])
            if qc == 1 and mt % 2 == 1:
                po2 = p_sc.tile([P, 2, 512], F32, tag="scps",
                                name=f"po2_{qc}_{mt}")
                po = po2[:, 0, :]
            else:
                pot = p_ps.tile([P, 512], F32, tag="psg",
                                name=f"po_{qc}_{mt}")
                po = pot[:]
            steps = [(0, r8), (1, r8), (0, r8r)]
            for si, (r, src) in enumerate(steps):
                for u in range(FT // 2):
                    nc.tensor.matmul(po, slab[:, 2 * u:2 * u + 2, r, :],
                                     src[:, 2 * u:2 * u + 2, :],
                                     start=(si == 0 and u == 0), stop=False,
                                     perf_mode=DRM)
            nc.tensor.matmul(po, sb_w2id[:, mt, :], f8_t[qc][:, mt, :],
                             start=False, stop=False)
            nc.tensor.matmul(po, sb_w2id[:, mt, :], f8r_t[qc][:, mt, :],
                             start=False, stop=True)
            ot = p_fo.tile([P, 512], F32, tag="ot")
            nc.vector.tensor_scalar(ot[:], po, 1.0 / 2048.0,
                                    sb_b2[:, mt:mt + 1], OP.mult, OP.add)
            nc.sync.dma_start(OUT_d[:, mt, bass.ts(qc, 512)], ot[:])
        return go

    _mark(nc, "y0/ln2")
    y_ln2(0)

    # =========================================================
    # A1: attention(qc=1), hiding the full FFN of qc=0
    # =========================================================
    q1 = [f1unit(0, ft) for ft in range(FT)] + [f2unit(0, mt)
                                               for mt in range(DT)]
    cum1 = [6, 12, 18, 24, 30, 34, 37, 40]
    sreq1 = [0] * DT
    drain1 = make_drain(q1, cum1, sreq1)
    for t in range(DT):
        _mark(nc, f"A1.t{t}")
        attention_step(1, t, drain1)
    while q1:
        q1.pop(0)()

    _mark(nc, "y1/ln2")
    y_ln2(1)
    _mark(nc, "ffn1-tail")
    for ft in range(FT):
        f1unit(1, ft)()
    _mark(nc, "ffn2-tail")
    for mt in range(DT):
        f2unit(1, mt)()
    _mark(nc, "end")

    for name in list(pools)[::-1]:
        close_pool(name)


def _prep_shared(inputs):
    """Host-side weight preprocessing (shared across cores)."""
    f32 = np.float32
    g1 = np.asarray(inputs["g1"], f32)
    beta1 = np.asarray(inputs["beta1"], f32)
    g2 = np.asarray(inputs["g2"], f32)
    beta2 = np.asarray(inputs["beta2"], f32)
    Wq = np.asarray(inputs["Wq"], f32)
    Wk = np.asarray(inputs["Wk"], f32)
    Wv = np.asarray(inputs["Wv"], f32)
    W1 = np.asarray(inputs["W1"], f32)
    W2 = np.asarray(inputs["W2"], f32)

    def fold(Wm, bm):
        Wp = Wm * g1[:, None]
        bp = np.asarray(inputs[bm], f32) + beta1 @ Wm
        return Wp, bp

    Wqp, bqp = fold(Wq, "bq")
    Wkp, bkp = fold(Wk, "bk")
    Wvp, bvp = fold(Wv, "bv")
    W1p = W1 * g2[:, None]
    b1p = np.asarray(inputs["b1"], f32) + beta2 @ W1
    b2p = np.asarray(inputs["b2"], f32) + beta2

    bf = ml_dtypes.bfloat16
    f8 = ml_dtypes.float8_e4m3

    def wtile(Wm, ntile, dtype):
        return np.ascontiguousarray(
            Wm.reshape(ntile, P, Wm.shape[1]).transpose(1, 0, 2)).astype(dtype)

    def wtile_split(Wm, ntile, scale):
        """fp8 value + fp8 residual of scale*Wm, packed [P, ntile, 2, cols]."""
        ws = (scale * Wm).astype(f32)
        a = ws.astype(f8)
        r = (ws - a.astype(f32)).astype(f8)
        at = wtile(a.astype(f32), ntile, f8)
        rt = wtile(r.astype(f32), ntile, f8)
        return np.ascontiguousarray(np.stack([at, rt], axis=2))

    def btile(bv_, ntile):
        return np.ascontiguousarray(bv_.reshape(ntile, P).T).astype(f32)

    E = np.zeros((2, DT, P), f32)
    for t in range(DT):
        for m in range(P):
            E[m // HD, t, m] = 1.0
    E = E.astype(bf)

    # identity (g2-diag) FFN2 term at the 32*64 combined scale, bf16
    w2id = np.zeros((P, DT, P), f32)
    for mt in range(DT):
        for p in range(P):
            w2id[p, mt, p] = 2048.0 * g2[mt * P + p]
    w2id = w2id.astype(bf)

    return {
        "wq": wtile(WSC * Wqp, DT, f8), "wk": wtile(WSC * Wkp, DT, f8),
        "wv": wtile(WSC * Wvp, DT, f8),
        "w1": wtile_split(W1p, DT, WSC),
        "w2a": wtile_split(W2, FT, 64.0),
        "w2id": w2id,
        "bq": btile(WSC * bqp, DT), "bk": btile(WSC * bkp, DT),
        "bvb": np.ascontiguousarray(
            np.broadcast_to(WSC * bvp, (P, D))).astype(f32),
        "b1": btile(WSC * b1p, FT), "b2": btile(b2p, DT),
        "emat": E,
    }


def _per_core_inputs(inputs, shared):
    x = np.asarray(inputs["x"], np.float32)
    f8 = ml_dtypes.float8_e4m3
    maps = []
    for c in range(NCORES):
        b, hf = c // 2, c % 2
        xTn = x[b].T.reshape(DT, P, S).transpose(1, 0, 2)
        if hf == 1:
            # roll so this core's own half is always columns 0:SQ
            xTn = np.concatenate([xTn[:, :, SQ:], xTn[:, :, :SQ]], axis=2)
        xTn = np.ascontiguousarray(xTn)
        m = dict(shared)
        m["xbf"] = xTn.astype(ml_dtypes.bfloat16)
        m["x8"] = xTn.astype(f8)
        m["xq8"] = (xTn.astype(np.float64) ** 2).astype(f8)
        m["xh"] = np.ascontiguousarray(xTn[:, :, :SQ])
        maps.append(m)
    return maps


def _get_sharded():
    """Build (once) the nc + jitted shard_map executable."""
    if "sharded" in _CACHE:
        return _CACHE["sharded"]

    import jax
    from jax.sharding import Mesh, PartitionSpec
    from jax.experimental.shard_map import shard_map
    from concourse import bass2jax
    from concourse import mybir as _mybir

    bass2jax.install_neuronx_cc_hook()
    nc = _build_nc()

    partition_name = (nc.partition_id_tensor.name
                      if nc.partition_id_tensor else None)
    in_names, out_names, out_avals, zero_shapes = [], [], [], []
    for alloc in nc.m.functions[0].allocations:
        if not isinstance(alloc, _mybir.MemoryLocationSet):
            continue
        name = alloc.memorylocations[0].name
        if alloc.kind == "ExternalInput":
            if name != partition_name:
                in_names.append(name)
        elif alloc.kind == "ExternalOutput":
            shape = tuple(alloc.tensor_shape)
            dtype = _mybir.dt.np(alloc.dtype)
            out_names.append(name)
            out_avals.append(jax.core.ShapedArray(shape, dtype))
            zero_shapes.append((shape, dtype))
    n_params = len(in_names)
    all_names = in_names + out_names
    if partition_name is not None:
        all_names = all_names + [partition_name]
    donate = tuple(range(n_params, n_params + len(out_names)))

    def _body(*args):
        operands = list(args)
        if partition_name is not None:
            operands.append(bass2jax.partition_id_tensor())
        outs = bass2jax._bass_exec_p.bind(
            *operands,
            out_avals=tuple(out_avals),
            in_names=tuple(all_names),
            out_names=tuple(out_names),
            lowering_input_output_aliases=(),
            sim_require_finite=True,
            sim_require_nnan=True,
            nc=nc,
        )
        return tuple(outs)

    devices = jax.devices()[:NCORES]
    mesh = Mesh(np.asarray(devices), ("core",))
    nin = n_params + len(out_names)
    sharded = jax.jit(
        shard_map(_body, mesh=mesh,
                  in_specs=(PartitionSpec("core"),) * nin,
                  out_specs=(PartitionSpec("core"),) * len(out_names),
                  check_rep=False),
        donate_argnums=donate, keep_unused=True)

    _CACHE["sharded"] = (nc, sharded, in_names, out_names, out_avals,
                         zero_shapes)
    return _CACHE["sharded"]


def _concat_inputs(in_maps):
    _, _, in_names, _, _, zero_shapes = _get_sharded()
    concat_in = [
        np.concatenate([np.asarray(in_maps[c][n]) for c in range(NCORES)],
                       axis=0)
        for n in in_names
    ]
    concat_zeros = [
        np.zeros((NCORES * s[0], *s[1:]), d) for (s, d) in zero_shapes
    ]
    return concat_in, concat_zeros


def _run(in_maps):
    nc, fn, in_names, out_names, out_avals, zero_shapes = _get_sharded()
    concat_in, concat_zeros = _concat_inputs(in_maps)
    outs = fn(*concat_in, *concat_zeros)
    res = []
    for c in range(NCORES):
        res.append({
            name: np.asarray(outs[i]).reshape(NCORES, *out_avals[i].shape)[c]
            for i, name in enumerate(out_names)
        })
    return res


def kernel(**inputs):
    shared = _prep_shared(inputs)
    in_maps = _per_core_inputs(inputs, shared)
    res = _run(in_maps)
    out = np.empty((B, S, D), np.float32)
    for c in range(NCORES):
        b, hf = c // 2, c % 2
        o = res[c]["OUT"]                       # [P, DT, SQ]
        out[b, hf * SQ:(hf + 1) * SQ, :] = o.transpose(2, 1, 0).reshape(SQ, D)
    return out
